# revision 1
# baseline (speedup 1.0000x reference)
"""LocallyConnected2d via TensorEngine scatter-matmul.

out[b,i,j] = bias[i,j] + sum_{u,v} x_pad[b, 2i+u, 2j+v] * w[i,j,u,v]

Mapping: shard output rows across 8 cores (14 rows/core, 33-row input slab).
Tile the slab into pixel-chunks of <=128 pixels (8-row x 16-col blocks, plus
a 1-row tail band). For each chunk:
  stationary lhsT = x^T [pixel, b]            (host-prepared gather)
  moving rhs     = scattered weights [pixel, f]  f = (i,j) window of chunk
  psum out[b, (i//4)*512 + (i%4)*112 + j]  accumulates over chunks (taps).
Bias is seeded into PSUM with a rank-1 ones x bias matmul (start=True), so
all chunk matmuls accumulate with start=False. Epilogue: per-bank PSUM->SBUF
cast-copy (fp16) then DMA out.
"""

import sys

sys.path.insert(0, "/opt/trn_rl_repo")

import numpy as np

import concourse.bass as bass
import concourse.bacc as bacc
import concourse.mybir as mybir
from concourse.tile import TileContext
from concourse.bass_utils import run_bass_kernel_spmd

B = 64
H = W = 224
KH = KW = 7
PH = PW = 3
NKH = NKW = 112
NCORES = 8
RPC = NKH // NCORES            # 14 output rows per core
SLAB = 2 * (RPC - 1) + KH      # 33 input rows per core
WP = W + 2 * PW                # 230 padded cols
OUTC = RPC * NKW               # 1568 output cols per core

F16 = mybir.dt.float16
F32 = mybir.dt.float32


def _chunk_table():
    chunks = []
    foff = 0
    bands = [(0, 8), (8, 8), (16, 8), (24, 8), (32, 1)]
    for r0, nr in bands:
        if nr == 8:
            cols = [(16 * k, 16) for k in range(14)] + [(224, 6)]
        else:
            cols = [(0, 128), (128, 102)]
        i0 = max(0, -((6 - r0) // 2))              # ceil((r0-6)/2)
        i1 = min(RPC - 1, (r0 + nr - 1) // 2)
        for c0, ncol in cols:
            j0 = max(0, -((6 - c0) // 2))
            j1 = min(NKW - 1, (c0 + ncol - 1) // 2)
            nj = j1 - j0 + 1
            fc = (i1 - i0 + 1) * nj
            chunks.append(dict(r0=r0, nr=nr, c0=c0, ncol=ncol,
                               i0=i0, i1=i1, j0=j0, j1=j1, nj=nj, foff=foff))
            foff += fc + (fc & 1)                  # keep 4B alignment
    return chunks, foff


CHUNKS, FTOT = _chunk_table()
NCH = len(CHUNKS)


def _mm_table():
    """Chunk matmuls, split so each targets a single PSUM bank (4 i's/bank).

    Banks 0,1 accumulate in PSUM partitions 0:64 (PE col-group 0); banks 2,3
    in partitions 64:128 (col-group 1). The two streams are interleaved so
    their LDWEIGHTS+MATMUL pairs run concurrently on the PE array halves.
    """
    lo, up = [], []
    for ci, ch in enumerate(CHUNKS):
        for bank in range(ch["i0"] // 4, ch["i1"] // 4 + 1):
            ia = max(ch["i0"], 4 * bank)
            ib = min(ch["i1"], 4 * bank + 3)
            (lo if bank < 2 else up).append([ci, ia, ib, bank, False])
    # merge the two streams evenly
    mms, il, iu = [], 0, 0
    while il < len(lo) or iu < len(up):
        if iu >= len(up) or (il < len(lo)
                             and il * len(up) <= iu * len(lo)):
            mms.append(lo[il]); il += 1
        else:
            mms.append(up[iu]); iu += 1
    last = {}
    for k, mm in enumerate(mms):
        last[mm[3]] = k
    for k in last.values():
        mms[k][4] = True                           # stop=True per bank
    return mms


MMS = _mm_table()


def _host_tables():
    # x gather: chunk pixel slot p -> (slab row, col), mask for unused slots
    gr = np.zeros((NCH, 128), np.int64)
    gc = np.zeros((NCH, 128), np.int64)
    gm = np.zeros((NCH, 128), np.float32)
    for ci, ch in enumerate(CHUNKS):
        p = np.arange(128)
        dr, dc = p // ch["ncol"], p % ch["ncol"]
        ok = dr < ch["nr"]
        gr[ci] = np.where(ok, ch["r0"] + dr, 0)
        gc[ci] = np.where(ok, ch["c0"] + dc, 0)
        gm[ci] = ok
    # weight scatter: (i,j,u,v) -> (pixel slot, f col)
    i = np.arange(RPC)[:, None, None, None]
    j = np.arange(NKW)[None, :, None, None]
    u = np.arange(KH)[None, None, :, None]
    v = np.arange(KW)[None, None, None, :]
    sh = (RPC, NKW, KH, KW)
    r = np.broadcast_to(2 * i + u, sh)             # slab row 0..32
    cc = np.broadcast_to(2 * j + v, sh)            # slab col 0..228
    kb = np.minimum(r // 8, 4)
    chunk_id = np.where(kb < 4,
                        kb * 15 + np.minimum(cc // 16, 14),
                        60 + (cc >= 128).astype(np.int64))
    def arr(key):
        return np.array([ch[key] for ch in CHUNKS])
    pp = ((r - arr("r0")[chunk_id]) * arr("ncol")[chunk_id]
          + (cc - arr("c0")[chunk_id]))
    ff = (arr("foff")[chunk_id]
          + (np.broadcast_to(i, sh) - arr("i0")[chunk_id]) * arr("nj")[chunk_id]
          + (np.broadcast_to(j, sh) - arr("j0")[chunk_id]))
    return gr, gc, gm, pp, ff


_TABLES = _host_tables()


def _shard_inputs(x, weights, bias):
    x = np.asarray(x, dtype=np.float32)
    weights = np.asarray(weights, dtype=np.float32)
    bias = np.asarray(bias, dtype=np.float32)
    gr, gc, gm, pp, ff = _TABLES

    xp = np.zeros((B, H + 2 * PH, WP), np.float32)
    xp[:, PH:PH + H, PW:PW + W] = x

    ii, jj = np.meshgrid(np.arange(RPC), np.arange(NKW), indexing="ij")
    bpos = (ii // 4) * 512 + (ii % 4) * 112 + jj

    in_maps = []
    for c in range(NCORES):
        slab = xp[:, 2 * RPC * c: 2 * RPC * c + SLAB, :]   # [64, 33, 230]
        g = slab[:, gr, gc] * gm                           # [64, NCH, 128]
        xt = np.ascontiguousarray(g.transpose(2, 1, 0)).astype(np.float16)
        wsc = np.zeros((128, FTOT), np.float16)
        wcore = weights[RPC * c: RPC * (c + 1)].astype(np.float16)
        wsc[pp.ravel(), ff.ravel()] = wcore.ravel()
        bc = np.zeros((1, 2048 + B), np.float16)
        bc[0, bpos.ravel()] = bias[RPC * c: RPC * (c + 1)].astype(np.float16).ravel()
        bc[0, 2048:] = 1.0                         # ones for the bias matmul
        in_maps.append({"xt": xt, "w": wsc, "bc": bc})
    return in_maps


def _one_iter(nc, pool, ppool, xt_d, w_d, b_d, o_d, mode="full"):
    xt = pool.tile([128, NCH, B], F16, tag="xt")
    wt = pool.tile([128, FTOT], F16, tag="wt")
    bt = pool.tile([1, 2048 + B], F16, tag="bt")
    ot = pool.tile([128, 896], F16, tag="ot")
    ps = ppool.tile([128, 2048], F32, tag="ps")

    if mode != "pe":
        # two big loads on separate HWDGE rings so their fixed costs overlap;
        # tiny bias load first so PSUM seeding can start immediately
        nc.sync.dma_start(out=bt[:, :], in_=b_d.ap())
        nc.sync.dma_start(out=xt[:, :, :], in_=xt_d.ap())
        nc.scalar.dma_start(out=wt[:, :], in_=w_d.ap())
    if mode == "dma":
        nc.gpsimd.dma_start(out=o_d.ap(), in_=ot[:, :])
        return

    # seed PSUM with bias (rank-1 ones x bias), start=True
    for bank in range(4):
        p0 = 0 if bank < 2 else B
        nc.tensor.matmul(
            out=ps[p0:p0 + B, 512 * bank: 512 * (bank + 1)],
            lhsT=bt[:, 2048:2048 + B],
            rhs=bt[:, 512 * bank: 512 * (bank + 1)],
            start=True, stop=False, skip_group_check=True)

    # accumulate all taps chunk by chunk
    for ci, ia, ib, bank, is_last in MMS:
        ch = CHUNKS[ci]
        ni = ib - ia + 1
        nj = ch["nj"]
        f0 = ch["foff"] + (ia - ch["i0"]) * nj
        off = 512 * bank + (ia - 4 * bank) * 112 + ch["j0"]
        p0 = 0 if bank < 2 else B
        base = ps[p0:p0 + B, off:off + 1]
        out_ap = bass.AP(tensor=base.tensor, offset=base.offset,
                         ap=[base.ap[0], [112, ni], [1, nj]])
        nc.tensor.matmul(
            out=out_ap,
            lhsT=xt[:, ci, :],
            rhs=wt[:, f0: f0 + ni * nj],
            start=False, stop=is_last, skip_group_check=True)

    # PSUM -> SBUF fp16 per bank (banks 2,3 live in the upper
    # partition half), then store both halves
    for bank in range(4):
        ilo = 4 * bank
        ncols = (min(RPC - 1, ilo + 3) - ilo + 1) * NKW
        p0 = 0 if bank < 2 else B
        c0 = 0 if bank % 2 == 0 else 448
        if bank % 2 == 0:
            nc.vector.tensor_copy(
                out=ot[p0:p0 + B, c0:c0 + ncols],
                in_=ps[p0:p0 + B, 512 * bank: 512 * bank + ncols])
        else:
            nc.scalar.copy(
                out=ot[p0:p0 + B, c0:c0 + ncols],
                in_=ps[p0:p0 + B, 512 * bank: 512 * bank + ncols])
    nc.gpsimd.dma_start(out=o_d.ap(), in_=ot[:, :])



def _build_nc(n_unroll=1, loop=None, mode="full", stagger=False):
    """Build the kernel program.

    n_unroll: python-unrolled iterations in the (loop) body.
    loop: if set, wrap the body in a hardware For_i loop with `loop` trips.
    Total iterations executed = n_unroll * (loop or 1).
    mode: "full" | "dma" (loads+store only) | "pe" (compute only) ablations.
    """
    nc = bacc.Bacc("TRN2", target_bir_lowering=False, debug=False,
                   num_devices=NCORES)
    xt_d = nc.dram_tensor("xt", [128, NCH, B], F16, kind="ExternalInput")
    w_d = nc.dram_tensor("w", [128, FTOT], F16, kind="ExternalInput")
    b_d = nc.dram_tensor("bc", [1, 2048 + B], F16, kind="ExternalInput")
    o_d = nc.dram_tensor("o", [2 * B, 896], F16, kind="ExternalOutput")

    with TileContext(nc) as tc:
        with tc.tile_pool(name="pool", bufs=3) as pool, \
             tc.tile_pool(name="ppool", bufs=2, space="PSUM") as ppool:
            if loop is None:
                for it in range(n_unroll):
                    _one_iter(nc, pool, ppool, xt_d, w_d, b_d, o_d, mode)
            else:
                with tc.For_i(0, loop, staggered_reset=stagger):
                    for it in range(n_unroll):
                        _one_iter(nc, pool, ppool, xt_d, w_d, b_d, o_d, mode)

    nc.compile()
    return nc


def _unshard_output(results):
    # o is [128, 896]: rows 0:64 = b -> i 0..7, rows 64:128 = b -> i 8..13
    cores = []
    for r in results:
        o = np.asarray(r["o"], np.float32)
        lo = o[:B, :8 * NKW].reshape(B, 8, NKW)
        hi = o[B:, :(RPC - 8) * NKW].reshape(B, RPC - 8, NKW)
        cores.append(np.concatenate([lo, hi], axis=1))   # [64, 14, 112]
    o = np.stack(cores)                                  # [8, 64, 14, 112]
    return np.ascontiguousarray(o.transpose(1, 0, 2, 3)).reshape(B, NKH, NKW)


def make_runner(nc, in_maps):
    """Build a cached jitted runner for nc; returns (run, unpack)."""
    import jax
    from jax.sharding import Mesh, PartitionSpec
    from jax.experimental.shard_map import shard_map
    from concourse.bass2jax import (_bass_exec_p, install_neuronx_cc_hook,
                                    partition_id_tensor)

    install_neuronx_cc_hook()
    n_cores = len(in_maps)
    partition_name = (nc.partition_id_tensor.name
                      if nc.partition_id_tensor else None)
    in_names, out_names, out_avals, zero_outs = [], [], [], []
    for alloc in nc.m.functions[0].allocations:
        if not isinstance(alloc, mybir.MemoryLocationSet):
            continue
        name = alloc.memorylocations[0].name
        if alloc.kind == "ExternalInput":
            if name != partition_name:
                in_names.append(name)
        elif alloc.kind == "ExternalOutput":
            shape = tuple(alloc.tensor_shape)
            dtype = mybir.dt.np(alloc.dtype)
            out_names.append(name)
            out_avals.append(jax.core.ShapedArray(shape, dtype))
            zero_outs.append(np.zeros(shape, dtype))
    n_params = len(in_names)
    all_in_names = list(in_names) + list(out_names)
    if partition_name is not None:
        all_in_names.append(partition_name)

    def _body(*args):
        operands = list(args)
        if partition_name is not None:
            operands.append(partition_id_tensor())
        return tuple(_bass_exec_p.bind(
            *operands, out_avals=tuple(out_avals),
            in_names=tuple(all_in_names), out_names=tuple(out_names),
            lowering_input_output_aliases=(), sim_require_finite=True,
            sim_require_nnan=True, nc=nc))

    devices = jax.devices()[:n_cores]
    mesh = Mesh(np.asarray(devices), ("core",))
    n_outs = len(out_names)
    sharded = jax.jit(
        shard_map(_body, mesh=mesh,
                  in_specs=(PartitionSpec("core"),) * (n_params + n_outs),
                  out_specs=(PartitionSpec("core"),) * n_outs,
                  check_rep=False),
        donate_argnums=tuple(range(n_params, n_params + n_outs)),
        keep_unused=True)

    concat_in = [np.concatenate([np.asarray(in_maps[c][nm])
                                 for c in range(n_cores)], axis=0)
                 for nm in in_names]
    concat_in = [jax.device_put(a) for a in concat_in]

    def run():
        zeros = [np.zeros((n_cores * z.shape[0], *z.shape[1:]), z.dtype)
                 for z in zero_outs]
        outs = sharded(*concat_in, *zeros)
        jax.block_until_ready(outs)
        return outs

    def unpack(outs):
        return [{nm: np.asarray(outs[i]).reshape(n_cores, *out_avals[i].shape)[c]
                 for i, nm in enumerate(out_names)} for c in range(n_cores)]

    return run, unpack


_NC_CACHE = None


def _get_nc():
    global _NC_CACHE
    if _NC_CACHE is None:
        _NC_CACHE = _build_nc()
    return _NC_CACHE


def kernel(x, weights, bias):
    nc = _get_nc()
    in_maps = _shard_inputs(x, weights, bias)
    res = run_bass_kernel_spmd(nc, in_maps, core_ids=list(range(NCORES)))
    return _unshard_output(res.results)


def benchmark(x, weights, bias, n_unroll=16, loop_big=512, reps=15):
    """Per-iteration HW time via wall-clock delta between a hardware-looped
    NEFF running n_unroll*loop_big iterations and one running n_unroll."""
    import time

    in_maps = _shard_inputs(x, weights, bias)
    n_small = n_unroll
    n_big = n_unroll * loop_big
    nc1 = _build_nc(n_unroll, loop=1)
    run1, _ = make_runner(nc1, in_maps)
    ncN = _build_nc(n_unroll, loop=loop_big)
    runN, unpackN = make_runner(ncN, in_maps)
    run1(); outsN = runN()
    t1, tN = [], []
    for _ in range(reps):
        t0 = time.perf_counter(); run1(); t1.append(time.perf_counter() - t0)
        t0 = time.perf_counter(); runN(); tN.append(time.perf_counter() - t0)
    times = {n_small: min(t1), n_big: min(tN)}
    per_iter_ns = (times[n_big] - times[n_small]) / (n_big - n_small) * 1e9
    return per_iter_ns, times, _unshard_output(unpackN(outsN))

